# revision 43
# baseline (speedup 1.0000x reference)
"""GroupedQueryAttention Trainium2 kernel (v2).

B=2, S=2048, D_MODEL=2048, 32 query heads / 8 KV heads, d_k=64.
Sharding: 8 cores = 2 (batch) x 4 (head groups of 8 query heads / 2 KV heads).
Per core: Wq/Wk/Wv column shard, Wo row shard; host sums the 4 partial
outputs per batch (the "all-reduce" of the row-parallel output projection).

v2 changes vs v1 (baseline 623us):
 - reciprocal_approx_fast (custom DVE op, ~5x faster than reciprocal) for
   the softmax denominators; the denominator still rides the ctx matmul's
   M dimension ([V dims (64) | ones (64)] lhsT -> rows 64-127 hold the
   replicated denominator) which costs zero extra PE cycles (cost ~ N).
 - deep cross-phase pipelining: Q projections for the next query tile and
   the output projection are emitted as PE gap-filler inside the
   attention stream; PSUM budgeted to exactly 8 banks so attention,
   projections and output projection overlap without collisions.
 - f16 output partials (halves output DMA).

fp8 (DoubleRow) was tried and abandoned: attention output is a weighted
average of near-zero-mean V, so weight/V quantization noise passes through
at full relative magnitude (measured 3.2e-2 in CoreSim, over the 2e-2 gate).
"""

import sys

sys.path.insert(0, "/opt/trn_rl_repo")

import numpy as np

import concourse.bass as bass
import concourse.tile as tile
from concourse import bacc, mybir
from concourse.bass_utils import run_bass_kernel_spmd
from concourse.masks import make_identity

F32 = mybir.dt.float32
F16 = mybir.dt.float16

D = 2048          # d_model
S = 2048          # sequence length
HL = 8            # query heads per core
KVL = 2           # kv heads per core
DK = 64
QO = HL * DK      # 512 query outdims per core
KO = KVL * DK     # 128 kv outdims per core
NKT = 16          # d_model contraction tiles of 128
NTT = 16          # token tiles of 128
NQT = 4           # query tiles of 512
NG = 8            # key-tile pairs (groups of 256 keys) per query tile
RC0 = 0.23583660820306038  # f16 NOT-seed reciprocal scale (y0 = -RC0*~den)
RC1 = 2.002                # first Newton constant (tuned for the f16 seed)

_CACHE = {}


def _build_nc():
    nc = bacc.Bacc("TRN2", target_bir_lowering=False)

    # host-prepacked layouts: contiguous per-partition runs on both DMA sides
    xT_h = nc.dram_tensor("xT", [128, 4, NKT, 512], F16, kind="ExternalInput")
    wq_h = nc.dram_tensor("wq", [128, 4, NKT, 128], F16, kind="ExternalInput")
    wk_h = nc.dram_tensor("wk", [128, NKT, KO], F16, kind="ExternalInput")
    wv_h = nc.dram_tensor("wv", [128, NKT, KO], F16, kind="ExternalInput")
    wo_h = nc.dram_tensor("wo", [QO, D], F16, kind="ExternalInput")
    bq_h = nc.dram_tensor("bq2", [128, 4], F32, kind="ExternalInput")
    bk_h = nc.dram_tensor("bk2", [128, 1], F32, kind="ExternalInput")
    bv_h = nc.dram_tensor("bv2", [128, 1], F32, kind="ExternalInput")
    out_h = nc.dram_tensor("out", [S, D], F16, kind="ExternalOutput")

    with tile.TileContext(nc) as tc:
        _emit(nc, tc, xT_h, wq_h, wk_h, wv_h, wo_h, bq_h, bk_h, bv_h, out_h)
    nc.compile()
    return nc


def _emit(nc, tc, xT_h, wq_h, wk_h, wv_h, wo_h, bq_h, bk_h, bv_h, out_h):
    from contextlib import ExitStack

    ctx = ExitStack()
    with ctx:
        consts = ctx.enter_context(tc.tile_pool(name="consts", bufs=1))
        persist = ctx.enter_context(tc.tile_pool(name="persist", bufs=1))
        # PSUM budget (8 banks): sp 2x2 + ctx 2 + pp 2
        spp = ctx.enter_context(tc.tile_pool(name="spp", bufs=2, space="PSUM"))
        ctxp = ctx.enter_context(tc.tile_pool(name="ctxp", bufs=1, space="PSUM"))
        ppp = ctx.enter_context(tc.tile_pool(name="ppp", bufs=2, space="PSUM"))
        ep = ctx.enter_context(tc.tile_pool(name="ep", bufs=6))
        rp = ctx.enter_context(tc.tile_pool(name="rp", bufs=4))
        nw = ctx.enter_context(tc.tile_pool(name="nw", bufs=2))
        op = ctx.enter_context(tc.tile_pool(name="op", bufs=4))

        ident = consts.tile([128, 128], F16)
        make_identity(nc, ident)

        # persistent SBUF
        wq_sb = persist.tile([128, 4, NKT, 128], F16)  # m-tile major
        wk_sb = persist.tile([128, NKT, KO], F16)
        wv_sb = persist.tile([128, NKT, KO], F16)
        wo_sb = persist.tile([128, 4, D], F16)
        xts = [persist.tile([128, NKT, 512], F16, name=f"xt{i}") for i in range(4)]
        qt_sb = persist.tile([128, 4, S], F16)     # [dim-in-pair, pair, token]
        ktd_sb = persist.tile([128, KVL, S], F16)  # kv dims dup'd both halves
        vt_sb = persist.tile([128, S], F16)        # [kv dims (2x64), token]
        # ctx lhsT: [token-in-128, key tile, kv, 64 V dims | 64 ones]
        vv_sb = persist.tile([128, NTT, KVL, 128], F16)
        ctxT_sb = persist.tile([128, 4, S], F16)   # [dim-in-pair, pair, token]

        bq_sb = consts.tile([128, 4], F32)
        bk_sb = consts.tile([128, 1], F32)
        bv_sb = consts.tile([128, 1], F32)

        # input DMAs, in consumption order (xt0/wk/wq0 first: they gate the
        # K projection and the attention ramp); all contiguous both sides
        nc.sync.dma_start(out=xts[0], in_=xT_h[:, 0])
        nc.sync.dma_start(out=wk_sb, in_=wk_h[:])
        nc.sync.dma_start(out=wq_sb[:, 0], in_=wq_h[:, 0])
        nc.sync.dma_start(out=bq_sb, in_=bq_h[:])
        nc.sync.dma_start(out=bk_sb, in_=bk_h[:])
        nc.sync.dma_start(out=bv_sb, in_=bv_h[:])
        nc.sync.dma_start(out=wv_sb, in_=wv_h[:])
        nc.sync.dma_start(out=xts[1], in_=xT_h[:, 1])
        nc.sync.dma_start(out=wq_sb[:, 1], in_=wq_h[:, 1])
        nc.sync.dma_start(out=xts[2], in_=xT_h[:, 2])
        nc.sync.dma_start(out=wq_sb[:, 2], in_=wq_h[:, 2])
        nc.sync.dma_start(out=xts[3], in_=xT_h[:, 3])
        nc.sync.dma_start(out=wq_sb[:, 3], in_=wq_h[:, 3])
        nc.sync.dma_start(out=wo_sb, in_=wo_h.rearrange("(c p) d -> p c d", p=128))

        nc.vector.memset(vv_sb, 1.0)  # ones cols; dim cols overwritten below

        # ---------------- projection pieces (emitted interleaved) ----------
        def k_proj(nt):
            ns = slice(nt * 512, (nt + 1) * 512)
            ps = ppp.tile([128, 512], F32, tag="pp")
            for kt in range(NKT):
                nc.tensor.matmul(
                    ps, lhsT=wk_sb[:, kt, :], rhs=xts[nt][:, kt, :],
                    start=(kt == 0), stop=(kt == NKT - 1))
            kb = rp.tile([128, 512], F16, tag="kb")
            nc.vector.tensor_scalar_add(kb, ps, bk_sb[:, 0:1])
            for kv in range(KVL):
                src = kb[kv * 64:(kv + 1) * 64, 0:512]
                nc.vector.tensor_copy(ktd_sb[0:64, kv, ns], src)
                nc.vector.tensor_copy(ktd_sb[64:128, kv, ns], src)

        def v_proj(nt):
            ns = slice(nt * 512, (nt + 1) * 512)
            ps = ppp.tile([128, 512], F32, tag="pp")
            for kt in range(NKT):
                nc.tensor.matmul(
                    ps, lhsT=wv_sb[:, kt, :], rhs=xts[nt][:, kt, :],
                    start=(kt == 0), stop=(kt == NKT - 1))
            nc.vector.tensor_scalar_add(vt_sb[:, ns], ps, bv_sb[:, 0:1])
            # V^T -> natural V layout via PE transpose into the ctx lhsT
            for tt in range(4 * nt, 4 * nt + 4):
                pst = ppp.tile([128, 512], F16, tag="pp", name="pst")
                nc.tensor.transpose(
                    pst[:, 0:128], vt_sb[:, tt * 128:(tt + 1) * 128], ident[:])
                for kv in range(KVL):
                    nc.vector.tensor_copy(
                        vv_sb[:, tt, kv, 0:64],
                        pst[:, kv * 64:(kv + 1) * 64])

        def q_proj(qt, mt):
            ns = slice(qt * 512, (qt + 1) * 512)
            ps = ppp.tile([128, 512], F32, tag="pp")
            for kt in range(NKT):
                nc.tensor.matmul(
                    ps, lhsT=wq_sb[:, mt, kt, :],
                    rhs=xts[qt][:, kt, :],
                    start=(kt == 0), stop=(kt == NKT - 1))
            nc.vector.tensor_scalar_add(qt_sb[:, mt, ns], ps, bq_sb[:, mt:mt + 1])

        def o_proj(qt, tls=range(4)):
            for tl in tls:
                ts_ = slice(qt * 512 + tl * 128, qt * 512 + (tl + 1) * 128)
                for dn in range(4):
                    ds_ = slice(dn * 512, (dn + 1) * 512)
                    ps = ppp.tile([128, 512], F32, tag="pp")
                    for c in range(4):
                        nc.tensor.matmul(
                            ps, lhsT=ctxT_sb[:, c, ts_], rhs=wo_sb[:, c, ds_],
                            start=(c == 0), stop=(c == 3))
                    ob = op.tile([128, 512], F16, tag="ob")
                    nc.vector.tensor_copy(ob, ps)
                    nc.sync.dma_start(out=out_h[ts_, ds_], in_=ob)

        # ------------- attention pair: 16 key tiles, 2 heads -------------
        def attn_pair(qt, pair, fill=None, fill_post=None):
            qs = slice(qt * 512, (qt + 1) * 512)
            kv = pair // 2
            ctx_ps = [
                ctxp.tile([128, 512], F32, tag=f"ctx{i}", name=f"ctx{i}")
                for i in range(2)
            ]
            # ctx matmuls are emitted `delay` key-tiles behind scores/exp so
            # fill work can slot between exp(kt) and ctx(kt) (accumulation
            # order in the PE queue is preserved; the sum is order-free)
            # ctx one key-tile behind scores/exp: when the PE FIFO reaches
            # ctx(kt), exp(kt) finished a window ago - no head-of-line wait
            delay = 2 if (fill or fill_post) else 1
            pend = []

            def emit_ctx(kt, e):
                for i in range(2):
                    nc.tensor.matmul(
                        ctx_ps[i][:, :],
                        lhsT=vv_sb[:, kt, kv, :],
                        rhs=e[:, i, :],
                        start=(kt == 0), stop=(kt == NTT - 1),
                    )

            for kt in range(NTT):
                if fill and kt in fill:
                    for f in fill[kt]:
                        f()
                sp = spp.tile([128, 2, 512], F32, tag="sp")
                ks = slice(kt * 128, (kt + 1) * 128)
                # both heads back-to-back on disjoint PE row groups
                for i in range(2):
                    nc.tensor.matmul(
                        sp[:, i, :],
                        lhsT=ktd_sb[i * 64:(i + 1) * 64, kv, ks],
                        rhs=qt_sb[i * 64:(i + 1) * 64, pair, qs],
                        start=True, stop=True,
                        tile_position=(i * 64, 0),
                    )
                e = ep.tile([128, 2, 512], F16, tag="e")
                nc.scalar.activation(
                    e[:, :, :], sp[:, :, :],
                    mybir.ActivationFunctionType.Exp, scale=0.125)
                if fill_post and kt in fill_post:
                    for f in fill_post[kt]:
                        f()
                pend.append((kt, e))
                if len(pend) > delay:
                    emit_ctx(*pend.pop(0))
            for args in pend:
                emit_ctx(*args)
            # evict PSUM fast (frees ctx banks), then normalize from SBUF.
            # 1/den via NOT-seeded Newton (6 short DVE ops per head instead of
            # one 3.3us RECIPROCAL that head-of-line-blocks the DVE FIFO).
            cus = []
            for i in range(2):
                cu = rp.tile([128, 512], F16, tag="cu")
                nc.vector.tensor_copy(cu, ctx_ps[i])
                cus.append(cu)
            MULT, ADD = mybir.AluOpType.mult, mybir.AluOpType.add
            for i in range(2):
                # realign den to base partition 0 (walrus requires equal SB
                # base partitions for two-tensor DVE ops); f16 copy runs 4x
                den = nw.tile([64, 512], F16, tag="den")
                nc.vector.tensor_copy(den, cus[i][64:128, :])
                n = nw.tile([64, 512], F16, tag="nt")
                # seed: bits(n) = ~bits(den); y0 = C0 * n approximates 1/den
                nc.vector.tensor_scalar(
                    n.bitcast(mybir.dt.int16), den.bitcast(mybir.dt.int16),
                    -1, None, mybir.AluOpType.bitwise_xor)
                s1 = nw.tile([64, 512], F16, tag="s1")
                nc.vector.scalar_tensor_tensor(s1, den, RC0, n, MULT, MULT)
                s2 = nw.tile([64, 512], F16, tag="s2")
                nc.vector.scalar_tensor_tensor(s2, s1, RC1, n, ADD, MULT)
                # y1 = (-RC0)*s2 ~ 1/den (one Newton step, ~2e-3 max err);
                # ctxT = ctx * y1
                nc.vector.scalar_tensor_tensor(
                    ctxT_sb[i * 64:(i + 1) * 64, pair, qs],
                    cus[i][0:64, :], -RC0, s2, MULT, MULT)

        # ------------- schedule: attention with interleaved fill ----------
        k_proj(0)
        q_proj(0, 0)
        v_proj(0)
        attn_pair(
            0, 0,
            fill={
                4: [lambda: k_proj(1)],
                8: [lambda: k_proj(2)],
                12: [lambda: k_proj(3)],
            },
            fill_post={
                4: [lambda: v_proj(1)],
                8: [lambda: v_proj(2)],
                12: [lambda: v_proj(3)],
            },
        )
        q_proj(0, 1)
        attn_pair(0, 1)
        q_proj(0, 2)
        q_proj(1, 0)
        attn_pair(0, 2)
        q_proj(0, 3)
        q_proj(1, 1)
        attn_pair(0, 3)
        q_proj(1, 2)
        for qt in range(1, NQT):
            attn_pair(qt, 0)
            q_proj(qt, 3)
            o_proj(qt - 1, tls=[0])
            attn_pair(qt, 1)
            if qt < 3:
                q_proj(qt + 1, 0)
            o_proj(qt - 1, tls=[1])
            attn_pair(qt, 2)
            if qt < 3:
                q_proj(qt + 1, 1)
            o_proj(qt - 1, tls=[2])
            attn_pair(qt, 3)
            if qt < 3:
                q_proj(qt + 1, 2)
            o_proj(qt - 1, tls=[3])
        # warmth bridge: the last pair's normalization (~5us of DVE latency)
        # gates every o_proj(3) matmul; a burst of throwaway matmuls keeps
        # the PE HAM clock at full rate so o_proj(3) doesn't run cold
        wps = spp.tile([128, 2, 512], F32, tag="sp")  # sp banks: attention is
        for _ in range(14):                           # done, never DVE-gated
            nc.tensor.matmul(wps[:, 0, :], lhsT=ident[:], rhs=qt_sb[:, 0, 0:512],
                             start=True, stop=True)
        o_proj(3)


def _get_nc():
    if "nc" not in _CACHE:
        _CACHE["nc"] = _build_nc()
    return _CACHE["nc"]


def _pack_x(x_b):
    """[S, D] -> [128, 4 slab, 16 kt, 512 t]: xT[k*128+p, nt*512+t]."""
    xT = x_b.T.astype(np.float16)                       # [D, S]
    return np.ascontiguousarray(
        xT.reshape(NKT, 128, 4, 512).transpose(1, 2, 0, 3))


def _pack_wq(wq_s):
    """[D, 512] -> [128, 4 mt, 16 kt, 128]: wq[k*128+p, mt*128+m]."""
    w = wq_s.astype(np.float16)
    return np.ascontiguousarray(
        w.reshape(NKT, 128, 4, 128).transpose(1, 2, 0, 3))


def _pack_wkv(w_s):
    """[D, 128] -> [128, 16 kt, 128]: w[k*128+p, m]."""
    w = w_s.astype(np.float16)
    return np.ascontiguousarray(w.reshape(NKT, 128, KO).transpose(1, 0, 2))


def kernel(x, Wq, bq, Wk, bk, Wv, bv, Wo, bo, _trace=False):
    x = np.asarray(x, np.float32)
    Wq = np.asarray(Wq, np.float32)
    bq = np.asarray(bq, np.float32)
    Wk = np.asarray(Wk, np.float32)
    bk = np.asarray(bk, np.float32)
    Wv = np.asarray(Wv, np.float32)
    bv = np.asarray(bv, np.float32)
    Wo = np.asarray(Wo, np.float32)
    bo = np.asarray(bo, np.float32)

    nc = _get_nc()
    in_maps = []
    for r in range(8):
        b, g = divmod(r, 4)
        qsl = slice(g * 512, (g + 1) * 512)
        ksl = slice(g * 128, (g + 1) * 128)
        in_maps.append({
            "xT": _pack_x(x[b]),
            "wq": _pack_wq(Wq[:, qsl]),
            "wk": _pack_wkv(Wk[:, ksl]),
            "wv": _pack_wkv(Wv[:, ksl]),
            "wo": np.ascontiguousarray(Wo[qsl, :].astype(np.float16)),
            "bq2": np.ascontiguousarray(bq[qsl].reshape(4, 128).T),
            "bk2": np.ascontiguousarray(bk[ksl].reshape(128, 1)),
            "bv2": np.ascontiguousarray(bv[ksl].reshape(128, 1)),
        })

    res = run_bass_kernel_spmd(nc, in_maps, list(range(8)), trace=_trace)
    out = np.zeros((2, S, D), np.float64)
    for r in range(8):
        out[r // 4] += res.results[r]["out"].astype(np.float64)
    out += bo.astype(np.float64)
    result = out.astype(np.float32)
    if _trace:
        return result, res
    return result
